# revision 48
# baseline (speedup 1.0000x reference)
"""Multi-head attention (B=4, S=1024, D=1024, H=16, DH=64) on 8 trn2 cores.

Tensor-parallel over heads: core c owns heads {2c, 2c+1}; each core runs
8 independent attention units (4 batches x 2 heads).  Per-head projections
only read a 64-channel slice of the input, so each core receives just its
2x64-channel slice, pre-transposed to [d, s] with a ones-row appended.

Math per unit (b, h), all biases/scales folded into packed weights
(E1 = 66: row/col 64 is the ones row, 65 zero pad for fp32r even dims):
  qT[e,s]   = WqTe.T @ xTe          (K=66: ones-row folds bias; 1/8 folded)
  kT[e,s]   = WkTe.T @ xTe
  v[t,e']   = xTe.T @ WvTe2         (col 64 == 1, bv folded into v)
  scT[t,s]  = kT.T @ qT             (scores transposed: t on partitions)
  expT      = exp(scT)              (no max-subtraction: |scores| <= ~10)
  outT[e',s]= sum_t v[t,e'] expT[t,s]   (row 64 = Z[s], softmax denominator)
  out[s,e]  = outT[e,s] / Z[s]      (transpose via identity matmul, then
                                     per-partition reciprocal multiply)

All matmuls run as float32r (same bits as fp32, ~5e-4 rel err, full PE
rate at moving-dim >= 256 vs 4 cyc/row for plain fp32).  The per-unit
scores/exp/PV loop is ACT(exp)-paced; projections of the next unit and
the epilogue of the previous unit are interleaved into it as filler
chunks, and the final PV t-block carries across unit boundaries so the
exp latency always hides behind PE work.
"""

import numpy as np

D = 1024
H = 16
DH = 64
B = 4
S = 1024
NCORES = 8
HPC = H // NCORES  # heads per core = 2
E1 = DH + 2  # 66: ones-row at 64, zero pad at 65 (fp32r wants even dims)
SCALE = 1.0 / np.sqrt(DH)

_CACHE = {}


def _split_sync_waits(nc, limit=1):
    """Walrus in this toolchain rejects instructions carrying more than one
    sync-wait; peel extra waits onto wait-only EventSemaphore ops inserted
    just before, on the same engine queue (engine streams are in-order)."""
    import concourse.mybir as mybir

    n = 0
    for bb in nc.main_func.blocks:
        out = []
        for ins in bb.instructions:
            si = ins.sync_info
            if si is not None and len(si.on_wait) > limit:
                waits = list(si.on_wait)
                for w in waits[:-limit]:
                    ev = mybir.InstEventSemaphore(
                        name=f"WSPLIT-{n}", ins=[], outs=[]
                    )
                    n += 1
                    ev.engine = ins.engine
                    ev.sync_info = mybir.SyncInfo(on_wait=[w], on_update=[])
                    out.append(ev)
                ins.sync_info = mybir.SyncInfo(
                    on_wait=waits[-limit:], on_update=list(si.on_update)
                )
            out.append(ins)
        bb.instructions = out
    return n


def _build_bass(split=True):
    import concourse.bass as bass
    import concourse.mybir as mybir
    import concourse.tile as tile

    f32 = mybir.dt.float32
    f32r = mybir.dt.float32r
    nc = bass.Bass()

    xTe_d = nc.declare_dram_parameter("xTe", [B, HPC, E1, S], f32r, isOutput=False)
    wq_d = nc.declare_dram_parameter("WqTe", [E1, HPC * DH], f32r, isOutput=False)
    wk_d = nc.declare_dram_parameter("WkTe", [E1, HPC * DH], f32r, isOutput=False)
    wv_d = nc.declare_dram_parameter("WvTe2", [E1, HPC * E1], f32r, isOutput=False)
    id_d = nc.declare_dram_parameter("ident", [E1, E1], f32r, isOutput=False)
    out_d = nc.declare_dram_parameter("out", [B, S, HPC, DH], f32, isOutput=True)

    with tile.TileContext(nc) as tc:
        with (
            tc.tile_pool(name="const", bufs=1) as constp,
            tc.tile_pool(name="sb", bufs=2) as sbp,
            tc.tile_pool(name="expp", bufs=2) as expp,
            tc.tile_pool(name="psA", bufs=2, space="PSUM") as psA,
            tc.tile_pool(name="psB", bufs=2, space="PSUM") as psB,
        ):
            wq_sb = constp.tile([E1, HPC * DH], f32r)
            wk_sb = constp.tile([E1, HPC * DH], f32r)
            wv_sb = constp.tile([E1, HPC * E1], f32r)
            id_sb = constp.tile([E1, E1], f32r)
            nc.gpsimd.dma_start(wq_sb[:], wq_d[:])
            nc.gpsimd.dma_start(wk_sb[:], wk_d[:])
            nc.gpsimd.dma_start(wv_sb[:], wv_d[:])
            nc.gpsimd.dma_start(id_sb[:], id_d[:])

            NT = S // 128  # 8 blocks of 128
            units = [(b, j) for b in range(B) for j in range(HPC)]

            xts = {}

            def fetch_xt(b, j):
                if (b, j) not in xts:
                    for jj in range(HPC):  # allocate both heads up front
                        xts[(b, jj)] = sbp.tile(
                            [E1, S], f32r, tag="xt", bufs=4,
                            name=f"xt_{b}_{jj}",
                        )
                    for half in range(2):  # first halves first: the first
                        for jj in range(HPC):  # qk chunks unblock sooner
                            hs = slice(half * 512, (half + 1) * 512)
                            nc.sync.dma_start(
                                xts[(b, jj)][:, hs], xTe_d[b, jj, :, hs]
                            )
                return xts[(b, j)]

            qk_pairs = {}

            def qk_pair_chunks(b):
                """qk_pair as 4 filler chunks (MM pair + copy each)."""
                qT = sbp.tile([128, S], f32r, tag="qT", bufs=3, name=f"qT_{b}")
                kT = sbp.tile([128, S], f32r, tag="kT", bufs=3, name=f"kT_{b}")
                chunks = []
                for sh in range(2):
                    for w_sb, dst in ((wq_sb, qT), (wk_sb, kT)):
                        def chunk(w_sb=w_sb, dst=dst, sh=sh):
                            # fp32r matmul output must start at psum
                            # partition 0; copy each head into its row band.
                            ss = slice(sh * 512, (sh + 1) * 512)
                            for j in range(HPC):
                                qk_ps = psB.tile(
                                    [128, 512], f32,
                                    tag="qkv" if j == 0 else "tr", bufs=1,
                                    name="qk_ps",
                                )
                                nc.tensor.matmul(
                                    qk_ps[:DH, :],
                                    w_sb[:, j * DH:(j + 1) * DH],
                                    fetch_xt(b, j)[:, ss],
                                    start=True, stop=True,
                                )
                                nc.vector.tensor_copy(
                                    dst[j * DH:(j + 1) * DH, ss],
                                    qk_ps[:DH, :],
                                )
                        chunks.append(chunk)
                return (qT, kT), chunks

            def v_chunks(b, j):
                """v projection as 2 filler chunks (4 MMs + copy each)."""
                xt = fetch_xt(b, j)
                v_sb = sbp.tile(
                    [128, NT * E1], f32r, tag="v", bufs=3, name=f"v_{b}_{j}"
                )
                chunks = []
                for half in range(2):
                    def chunk(half=half):
                        v_ps = psB.tile(
                            [128, 512], f32, tag="qkv", bufs=1, name="v_ps"
                        )
                        for q in range(4):
                            tb = half * 4 + q
                            nc.tensor.matmul(
                                v_ps[:, q * E1:(q + 1) * E1],
                                xt[:, tb * 128:(tb + 1) * 128],
                                wv_sb[:, j * E1:(j + 1) * E1],
                                start=True, stop=True,
                            )
                        nc.vector.tensor_copy(
                            v_sb[:, half * 4 * E1:(half + 1) * 4 * E1],
                            v_ps[:, :4 * E1],
                        )
                    chunks.append(chunk)
                return v_sb, chunks

            def pv_mms(ps, vs, tb, expT):
                for sh in range(2):
                    nc.tensor.matmul(
                        ps[:, sh * 512:(sh + 1) * 512],
                        vs[:, tb * E1:(tb + 1) * E1],
                        expT[:, sh * 512:(sh + 1) * 512],
                        start=(tb == 0), stop=(tb == NT - 1),
                    )

            pend = []  # FIFO of deferred PV t-blocks (depth 2)

            def scores_pv(b, j, qT, kT, v_sb, fillers, on_prev_done=None):
                """scores^T -> exp -> PV accumulation; returns PV psum.

                `fillers` is a deque of closures (next unit's projections,
                previous unit's epilogue blocks); up to two are emitted per
                t-block so PE/DVE always have independent work while ACT is
                the pacing engine.  `on_prev_done` fires right after the
                carried-over final PV matmul of the previous unit is emitted.
                """
                pv_ps = psA.tile(
                    [E1, S], f32, tag="pv", bufs=1, name=f"pv_{b}_{j}"
                )

                # PV for t-block tb is emitted after the scores matmuls of
                # tb+1 (carrying across unit boundaries), so the exp ACT
                # latency always hides behind PE work.
                for tb in range(NT):
                    sc_ps = psA.tile(
                        [128, S], f32, tag="sc", bufs=2, name="sc_ps"
                    )
                    for sh in range(2):
                        nc.tensor.matmul(
                            sc_ps[:, sh * 512:(sh + 1) * 512],
                            kT[:, tb * 128:(tb + 1) * 128],
                            qT[:, sh * 512:(sh + 1) * 512],
                            start=True, stop=True,
                        )
                    expT = expp.tile(
                        [128, S], f32r, tag="expT", bufs=6, name="expT"
                    )
                    nc.scalar.activation(
                        expT[:], sc_ps[:], mybir.ActivationFunctionType.Exp
                    )
                    if len(pend) >= 2:
                        item = pend.pop(0)
                        pv_mms(*item)
                        if (item[0] is not pv_ps and item[2] == NT - 1
                                and on_prev_done is not None):
                            on_prev_done()
                            on_prev_done = None
                    pend.append((pv_ps, v_sb, tb, expT))
                    # keep the final t-block slot light so the next unit's
                    # first scores matmul reaches the PE queue sooner
                    for _ in range(4 if tb < NT - 2 else 1):
                        if fillers:
                            fillers.popleft()()
                return pv_ps

            def epilogue_copies(b, j, pv_ps):
                """Drain PV psum to SBUF (frees the pv slot promptly)."""
                outT_sb = sbp.tile(
                    [E1, S], f32r, tag="outT", bufs=3, name=f"outT_{b}_{j}"
                )
                for blk in range(NT):
                    cs = slice(blk * 128, (blk + 1) * 128)
                    nc.vector.tensor_copy(outT_sb[:, cs], pv_ps[:, cs])
                return outT_sb

            def epilogue_chunks(b, j, outT_sb, pool=None, tag="tr", bufs=1):
                """Transpose + normalize + store, one chunk per 128-block."""
                chunks = []
                for blk in range(NT):
                    def chunk(blk=blk):
                        cs = slice(blk * 128, (blk + 1) * 128)
                        tr_ps = (pool or psB).tile(
                            [128, 512], f32, tag=tag, bufs=bufs, name="tr_ps"
                        )
                        nc.tensor.matmul(
                            tr_ps[:, :E1],
                            outT_sb[:, cs],
                            id_sb[:],
                            start=True, stop=True,
                        )
                        invz = sbp.tile(
                            [128, 1], f32, tag="invz", bufs=4, name="invz"
                        )
                        nc.vector.reciprocal(invz[:], tr_ps[:, DH:DH + 1])
                        o_sb = sbp.tile(
                            [128, DH], f32, tag="o", bufs=4, name="o_sb"
                        )
                        nc.vector.tensor_scalar_mul(
                            o_sb[:], tr_ps[:, :DH], invz[:]
                        )
                        nc.sync.dma_start(
                            out_d[b, blk * 128:(blk + 1) * 128, j, :], o_sb[:]
                        )
                    chunks.append(chunk)
                return chunks

            # Software pipeline across units: the next unit's projection
            # chunks and the previous unit's epilogue chunks are interleaved
            # into the current unit's ACT-paced scores loop as fillers.
            from collections import deque

            fillers = deque()

            def unit_inputs(idx):
                b, j = units[idx]
                if b not in qk_pairs:
                    fetch_xt(b, 0), fetch_xt(b, 1)
                    pair, chunks = qk_pair_chunks(b)
                    qk_pairs[b] = pair
                    for c in chunks:
                        fillers.append(c)
                qT, kT = qk_pairs[b]
                v_sb, vchunks = v_chunks(b, j)
                for c in vchunks:
                    fillers.append(c)
                return (
                    qT[j * DH:(j + 1) * DH, :],
                    kT[j * DH:(j + 1) * DH, :],
                    v_sb,
                )

            pending = unit_inputs(0)
            while fillers:  # unit 0's projections run before its loop
                fillers.popleft()()
            pv_prev = None
            for idx in range(len(units)):
                b, j = units[idx]
                nxt = unit_inputs(idx + 1) if idx + 1 < len(units) else None

                on_prev_done = None
                if pv_prev is not None:
                    prev_unit, prev_ps = units[idx - 1], pv_prev

                    def on_prev_done(prev_unit=prev_unit, prev_ps=prev_ps):
                        outT_prev = epilogue_copies(*prev_unit, prev_ps)
                        for c in reversed(epilogue_chunks(*prev_unit, outT_prev)):
                            fillers.appendleft(c)

                pv_prev = scores_pv(b, j, *pending, fillers, on_prev_done)
                pending = nxt
            while pend:
                pv_mms(*pend.pop(0))
            while fillers:
                fillers.popleft()()
            outT_last = epilogue_copies(*units[-1], pv_prev)
            for c in epilogue_chunks(
                *units[-1], outT_last, pool=psA, tag="sc", bufs=2
            ):
                c()
    if split:
        _split_sync_waits(nc)
    return nc


def _prep_inputs(sequences, Wq, Wk, Wv, bq, bk, bv):
    """Host-side packing: per-core input maps."""
    sequences = np.ascontiguousarray(np.asarray(sequences, dtype=np.float32))
    Wq = np.asarray(Wq, np.float32)
    Wk = np.asarray(Wk, np.float32)
    Wv = np.asarray(Wv, np.float32)
    bq = np.asarray(bq, np.float32)
    bk = np.asarray(bk, np.float32)
    bv = np.asarray(bv, np.float32)

    ident = np.eye(E1, dtype=np.float32)
    # [B, S, H, DH] -> [H, B, DH, S] transposed slices
    xT = np.ascontiguousarray(
        sequences.reshape(B, S, H, DH).transpose(2, 0, 3, 1)
    )  # [H, B, DH, S]

    in_maps = []
    for c in range(NCORES):
        heads = [HPC * c + j for j in range(HPC)]
        xTe = np.zeros((B, HPC, E1, S), np.float32)
        xTe[:, :, DH, :] = 1.0
        for j, h in enumerate(heads):
            xTe[:, j, :DH, :] = xT[h]
        wq = np.zeros((E1, HPC, DH), np.float32)
        wk = np.zeros((E1, HPC, DH), np.float32)
        wv = np.zeros((E1, HPC, E1), np.float32)
        for j, h in enumerate(heads):
            wq[:DH, j, :] = Wq[h].T * SCALE
            wq[DH, j, :] = bq[h] * SCALE
            wk[:DH, j, :] = Wk[h].T
            wk[DH, j, :] = bk[h]
            wv[:DH, j, :DH] = Wv[h].T
            wv[DH, j, :DH] = bv[h]
            wv[DH, j, DH] = 1.0  # ones column -> Z row of outT
        in_maps.append({
            "xTe": xTe,
            "WqTe": wq.reshape(E1, HPC * DH),
            "WkTe": wk.reshape(E1, HPC * DH),
            "WvTe2": wv.reshape(E1, HPC * E1),
            "ident": ident,
        })
    return in_maps


def get_nc():
    if "nc" not in _CACHE:
        _CACHE["nc"] = _build_bass()
    return _CACHE["nc"]


def kernel(sequences, Wq, Wk, Wv, bq, bk, bv):
    from concourse.bass_utils import run_bass_kernel_spmd

    nc = get_nc()
    in_maps = _prep_inputs(sequences, Wq, Wk, Wv, bq, bk, bv)
    res = run_bass_kernel_spmd(nc, in_maps, list(range(NCORES)))
    full = np.empty((B, S, D), np.float32)
    for c in range(NCORES):
        full[:, :, c * HPC * DH:(c + 1) * HPC * DH] = (
            res.results[c]["out"].reshape(B, S, HPC * DH)
        )
    return full
